# revision 1
# baseline (speedup 1.0000x reference)
"""Trainium2 Bass kernel for LoopConnectivityDecoder.

Math: out[i,j] (i<j) = sigmoid( sum_k W2[k] * relu(a'[i,k] + b'[k,j]) + b2 ),
symmetrized, zero diagonal; where a' = X@W1[:,:32].T + b1, b' = (X@W1[:,32:].T).T.

Device strategy (8 cores, SPMD):
- Fold the signed W2[k] scale into both halves on the host:
  z_k[i,j] = W2[k]*a'[i,k] + W2[k]*b'[k,j]; then W2[k]*relu(a'+b') =
  max(z,0) for W2[k]>=0 else min(z,0).
- The upper triangle is covered by 24 uniform (128 rows x 512 cols) units,
  3 per core, via host-side gathers (per-core packed operand tensors).
- Per unit, per k: PE does a K=2 matmul outer-sum (lhsT=[a-row;ones],
  rhs=[ones;b-row]) into PSUM; DVE then runs ONE fused
  scalar_tensor_tensor: acc = (y maxmin 0) + acc (64-op chain).
- Tail: ACT sigmoid with b2 bias; DMA the (128x512) score tile out.
- Host scatters unit tiles into the full matrix, applies triu, mirrors.
"""

import numpy as np

N = 1536
EMB = 32
H = 64
P = 128          # partition tile (rows per unit)
F = 512          # free-dim tile (cols per unit)
KC = 16          # k-chunk size for SBUF staging of packed operands
NCORES = 8
NBLK = N // P    # 12 row blocks
UNITS_PER_CORE = 3

_cache = {}


def _unit_list():
    """24 (row_block, col0) units covering the upper-triangle staircase."""
    units = []
    for bi in range(NBLK):
        cols = N - P * bi
        nch = -(-cols // F)
        for t in range(nch):
            col0 = min(P * bi + F * t, N - F)
            units.append((bi, col0))
    assert len(units) == NCORES * UNITS_PER_CORE
    return units


def _build_module(op_is_max):
    """Build + compile the Bass module. op_is_max: tuple of 64 bools."""
    from contextlib import ExitStack
    import concourse.tile as tile
    from concourse import bacc, mybir

    nc = bacc.Bacc("TRN2", target_bir_lowering=False, debug=False,
                   num_devices=NCORES)
    A1_d = nc.dram_tensor("A1g", [2, H, UNITS_PER_CORE * P], mybir.dt.float32,
                          kind="ExternalInput")
    B1_d = nc.dram_tensor("B1g", [2, H, UNITS_PER_CORE * F], mybir.dt.float32,
                          kind="ExternalInput")
    b2_d = nc.dram_tensor("b2c", [P, 1], mybir.dt.float32, kind="ExternalInput")
    out_d = nc.dram_tensor("out", [UNITS_PER_CORE, P, F], mybir.dt.float32,
                           kind="ExternalOutput")

    with tile.TileContext(nc) as tc, ExitStack() as ctx:
        const = ctx.enter_context(tc.tile_pool(name="const", bufs=1))
        ld = ctx.enter_context(tc.tile_pool(name="ld", bufs=3))
        accp = ctx.enter_context(tc.tile_pool(name="accp", bufs=2))
        outp = ctx.enter_context(tc.tile_pool(name="outp", bufs=2))
        psum = ctx.enter_context(tc.tile_pool(name="psum", bufs=6, space="PSUM"))

        b2_t = const.tile([P, 1], mybir.dt.float32)
        nc.sync.dma_start(b2_t[:], b2_d[:])

        for u in range(UNITS_PER_CORE):
            acc = None
            for c in range(H // KC):
                a_t = ld.tile([2, KC, P], mybir.dt.float32, tag="a")
                nc.sync.dma_start(
                    a_t[:], A1_d[:, c * KC:(c + 1) * KC, u * P:(u + 1) * P])
                b_t = ld.tile([2, KC, F], mybir.dt.float32, tag="b")
                nc.sync.dma_start(
                    b_t[:], B1_d[:, c * KC:(c + 1) * KC, u * F:(u + 1) * F])
                for kk in range(KC):
                    k = c * KC + kk
                    y = psum.tile([P, F], mybir.dt.float32, tag="y")
                    nc.tensor.matmul(y[:], a_t[0:2, kk, :], b_t[0:2, kk, :],
                                     start=True, stop=True)
                    op0 = (mybir.AluOpType.max if op_is_max[k]
                           else mybir.AluOpType.min)
                    newacc = accp.tile([P, F], mybir.dt.float32, tag="acc")
                    if acc is None:
                        nc.vector.tensor_scalar(newacc[:], y[:], 0.0, None, op0)
                    else:
                        nc.vector.scalar_tensor_tensor(
                            newacc[:], y[:], 0.0, acc[:], op0,
                            mybir.AluOpType.add)
                    acc = newacc
            s_t = outp.tile([P, F], mybir.dt.float32, tag="s")
            nc.scalar.activation(s_t[:], acc[:],
                                 mybir.ActivationFunctionType.Sigmoid,
                                 bias=b2_t[:, 0:1], scale=1.0)
            nc.sync.dma_start(out_d[u], s_t[:])

    nc.compile()
    return nc


def kernel(loop_embeddings, W1, b1, W2, b2):
    from concourse.bass_utils import run_bass_kernel_spmd

    X = np.asarray(loop_embeddings, dtype=np.float32)
    W1 = np.asarray(W1, dtype=np.float32)
    b1 = np.asarray(b1, dtype=np.float32)
    W2 = np.asarray(W2, dtype=np.float32)
    b2 = np.asarray(b2, dtype=np.float32)

    a = X @ W1[:, :EMB].T + b1          # (N, H)  a'[i,k]
    bm = X @ W1[:, EMB:].T              # (N, H)  b'[k,j] = bm[j,k]
    w2 = W2[0]                          # (H,)

    az = (w2[None, :] * a).T.copy()     # (H, N): w2[k]*a'[i,k]
    bz = (w2[None, :] * bm).T.copy()    # (H, N): w2[k]*b'[k,j]

    units = _unit_list()

    in_maps = []
    for core in range(NCORES):
        A1g = np.empty((2, H, UNITS_PER_CORE * P), dtype=np.float32)
        B1g = np.empty((2, H, UNITS_PER_CORE * F), dtype=np.float32)
        A1g[1] = 1.0
        B1g[0] = 1.0
        for u in range(UNITS_PER_CORE):
            bi, col0 = units[core * UNITS_PER_CORE + u]
            A1g[0, :, u * P:(u + 1) * P] = az[:, bi * P:(bi + 1) * P]
            B1g[1, :, u * F:(u + 1) * F] = bz[:, col0:col0 + F]
        in_maps.append({
            "A1g": A1g,
            "B1g": B1g,
            "b2c": np.full((P, 1), b2[0], dtype=np.float32),
        })

    key = tuple(bool(v) for v in (w2 >= 0))
    if key not in _cache:
        _cache[key] = _build_module(key)
    nc = _cache[key]

    res = run_bass_kernel_spmd(nc, in_maps, list(range(NCORES)))

    s = np.zeros((N, N), dtype=np.float32)
    for core in range(NCORES):
        o = res.results[core]["out"]
        for u in range(UNITS_PER_CORE):
            bi, col0 = units[core * UNITS_PER_CORE + u]
            s[bi * P:(bi + 1) * P, col0:col0 + F] = o[u]
    up = np.triu(s, 1)
    return (up + up.T).astype(np.float32)


# revision 4
# speedup vs baseline: 1.3894x; 1.3894x over previous
"""Trainium2 Bass kernel for LoopConnectivityDecoder.

Math: out[i,j] (i<j) = sigmoid( sum_k W2[k] * relu(a'[i,k] + b'[k,j]) + b2 ),
symmetrized, zero diagonal; a' = X@W1[:,:32].T + b1, b' = (X@W1[:,32:].T).T.

Device strategy (8 cores, SPMD, per-core work fixed by host-side gathers):
- Signed scale folded into data: z_k = W2[k]*a' + W2[k]*b'. Then
  W2[k]*relu(a'+b') = max(z,0) if W2[k]>=0 else min(z,0).
- Upper triangle covered by 24 uniform (128 x 512) units, 3 per core.
- Per k: one K=4 bf16 matmul computes the outer sum z in PSUM at full fp32
  accuracy via hi/lo bf16 splitting: lhsT=[a_hi;a_lo;1;1], rhs=[1;1;b_hi;b_lo].
- k's are sign-grouped and chunked by 4 (groups zero-padded to 4-multiples):
  4 matmuls fill a (128,4,512) PSUM tile; ScalarE drains it with one fused
  relu (scale=+/-1 by sign) into SBUF; VectorE/GpSimd run 4-wide interleaved
  accumulate chains (scalar_tensor_tensor: acc = staged*(+/-1) + acc).
- Tail per unit: merge chains, sigmoid(+b2) on ScalarE, DMA out.
- Host scatters unit tiles into the full matrix, applies triu, mirrors.
"""

import numpy as np
import ml_dtypes

N = 1536
EMB = 32
H = 64
P = 128          # partition tile (rows per unit)
F = 512          # free-dim tile (cols per unit)
NCORES = 8
NBLK = N // P    # 12 row blocks
UNITS_PER_CORE = 3
CH = 4           # k's per chunk (PSUM tile = CH banks)
LDG = 8          # k-slots per DMA load group

_cache = {}


def _unit_list():
    """24 (row_block, col0) units covering the upper-triangle staircase."""
    units = []
    for bi in range(NBLK):
        cols = N - P * bi
        nch = -(-cols // F)
        for t in range(nch):
            col0 = min(P * bi + F * t, N - F)
            units.append((bi, col0))
    assert len(units) == NCORES * UNITS_PER_CORE
    return units


def _slot_list(pos_mask):
    """Sign-grouped, zero-padded slot list.

    Returns (slots, chunk_signs): slots[i] is a k index or None (zero pad);
    chunk_signs[c] is +1/-1 for slots[4c:4c+4]."""
    pos = [k for k in range(H) if pos_mask[k]]
    neg = [k for k in range(H) if not pos_mask[k]]
    slots, signs = [], []
    for grp, sgn in ((pos, 1.0), (neg, -1.0)):
        if not grp:
            continue
        pad = (-len(grp)) % CH
        g = [None] * pad + grp
        slots += g
        signs += [sgn] * (len(g) // CH)
    assert len(slots) % CH == 0
    return slots, signs


def _build_module(pos_mask, repeat=1, n_dve_chunks=None):
    """Build + compile the Bass module. pos_mask: tuple of 64 bools."""
    from contextlib import ExitStack
    import concourse.tile as tile
    from concourse import bacc, mybir

    slots, signs = _slot_list(pos_mask)
    S = len(slots)
    NCH = S // CH
    NLD = -(-S // LDG)
    if n_dve_chunks is None:
        n_dve_chunks = max(1, min(NCH - 1, round(NCH * 11 / 17)))

    nc = bacc.Bacc("TRN2", target_bir_lowering=False, debug=False,
                   num_devices=NCORES)
    A1_d = nc.dram_tensor("A1g", [4, S, UNITS_PER_CORE * P], mybir.dt.bfloat16,
                          kind="ExternalInput")
    B1_d = nc.dram_tensor("B1g", [4, S, UNITS_PER_CORE * F], mybir.dt.bfloat16,
                          kind="ExternalInput")
    b2_d = nc.dram_tensor("b2c", [P, 1], mybir.dt.float32, kind="ExternalInput")
    out_d = nc.dram_tensor("out", [UNITS_PER_CORE, P, F], mybir.dt.float32,
                           kind="ExternalOutput")

    with tile.TileContext(nc) as tc, ExitStack() as ctx:
        const = ctx.enter_context(tc.tile_pool(name="const", bufs=1))
        ld = ctx.enter_context(tc.tile_pool(name="ld", bufs=4))
        stg = ctx.enter_context(tc.tile_pool(name="stg", bufs=4))
        accp = ctx.enter_context(tc.tile_pool(name="accp", bufs=2))
        outp = ctx.enter_context(tc.tile_pool(name="outp", bufs=2))
        psum = ctx.enter_context(tc.tile_pool(name="psum", bufs=2, space="PSUM"))

        b2_t = const.tile([P, 1], mybir.dt.float32)
        nc.sync.dma_start(b2_t[:], b2_d[:])

        def body():
            for u in range(UNITS_PER_CORE):
                a_tiles, b_tiles = [], []
                for g in range(NLD):
                    s0 = g * LDG
                    sw = min(LDG, S - s0)
                    a_t = ld.tile([4, LDG, P], mybir.dt.bfloat16, tag="a")
                    nc.sync.dma_start(
                        a_t[:, 0:sw], A1_d[:, s0:s0 + sw, u * P:(u + 1) * P])
                    b_t = ld.tile([4, LDG, F], mybir.dt.bfloat16, tag="b")
                    nc.sync.dma_start(
                        b_t[:, 0:sw], B1_d[:, s0:s0 + sw, u * F:(u + 1) * F])
                    a_tiles.append(a_t)
                    b_tiles.append(b_t)

                accD = accN = None
                for c in range(NCH):
                    sgn = signs[c]
                    y = psum.tile([P, CH, F], mybir.dt.float32, tag="y")
                    for q in range(CH):
                        s = c * CH + q
                        g, off = s // LDG, s % LDG
                        nc.tensor.matmul(y[:, q],
                                         a_tiles[g][0:4, off, :],
                                         b_tiles[g][0:4, off, :],
                                         start=True, stop=True)
                    t4 = stg.tile([P, CH, F], mybir.dt.float32, tag="t4")
                    nc.scalar.activation(t4[:], y[:],
                                         mybir.ActivationFunctionType.Relu,
                                         scale=float(sgn))
                    # accumulate: acc += sgn * t4 (4-wide interleaved chain)
                    on_dve = c < n_dve_chunks
                    if on_dve:
                        newacc = accp.tile([P, CH, F], mybir.dt.float32,
                                           tag="accD")
                        if accD is None:
                            nc.vector.tensor_scalar(newacc[:], t4[:],
                                                    float(sgn), None,
                                                    mybir.AluOpType.mult)
                        else:
                            nc.vector.scalar_tensor_tensor(
                                newacc[:], t4[:], float(sgn), accD[:],
                                mybir.AluOpType.mult, mybir.AluOpType.add)
                        accD = newacc
                    else:
                        # gpsimd: walrus rejects TensorScalarPtr on Pool, so
                        # chain with plain tensor_tensor add/subtract.
                        newacc = accp.tile([P, CH, F], mybir.dt.float32,
                                           tag="accN")
                        if accN is None:
                            accN = accp.tile([P, CH, F], mybir.dt.float32,
                                             tag="accN")
                            nc.gpsimd.memset(accN[:], 0.0)
                        op = (mybir.AluOpType.add if sgn > 0
                              else mybir.AluOpType.subtract)
                        nc.gpsimd.tensor_tensor(newacc[:], accN[:], t4[:], op)
                        accN = newacc

                # merge chains: logit = sum over 4 slices (+ gpsimd chain)
                lg = outp.tile([P, F], mybir.dt.float32, tag="lg")
                a2 = outp.tile([P, 2, F], mybir.dt.float32, tag="a2")
                nc.vector.tensor_tensor(a2[:], accD[:, 0:2], accD[:, 2:4],
                                        mybir.AluOpType.add)
                if accN is not None:
                    p2 = outp.tile([P, 2, F], mybir.dt.float32, tag="p2")
                    nc.gpsimd.tensor_tensor(p2[:], accN[:, 0:2], accN[:, 2:4],
                                            mybir.AluOpType.add)
                    nc.vector.tensor_tensor(a2[:], a2[:], p2[:],
                                            mybir.AluOpType.add)
                nc.vector.tensor_tensor(lg[:], a2[:, 0], a2[:, 1],
                                        mybir.AluOpType.add)
                s_t = outp.tile([P, F], mybir.dt.float32, tag="s")
                nc.scalar.activation(s_t[:], lg[:],
                                     mybir.ActivationFunctionType.Sigmoid,
                                     bias=b2_t[:, 0:1], scale=1.0)
                nc.sync.dma_start(out_d[u], s_t[:])

        if repeat > 1:
            with tc.For_i(0, repeat, 1):
                body()
        else:
            body()

    nc.compile()
    return nc


def _split_bf16(x):
    """Split fp32 array into (hi, lo) bf16 arrays with hi+lo ~= x."""
    hi = x.astype(ml_dtypes.bfloat16)
    lo = (x - hi.astype(np.float32)).astype(ml_dtypes.bfloat16)
    return hi, lo


def _prep_inputs(loop_embeddings, W1, b1, W2, b2):
    X = np.asarray(loop_embeddings, dtype=np.float32)
    W1 = np.asarray(W1, dtype=np.float32)
    b1 = np.asarray(b1, dtype=np.float32)
    W2 = np.asarray(W2, dtype=np.float32)
    b2 = np.asarray(b2, dtype=np.float32)

    a = X @ W1[:, :EMB].T + b1          # (N, H)
    bm = X @ W1[:, EMB:].T              # (N, H)
    w2 = W2[0]

    az = (w2[None, :] * a).T            # (H, N): z-contribution rows (i)
    bz = (w2[None, :] * bm).T           # (H, N): z-contribution rows (j)
    az_hi, az_lo = _split_bf16(az)
    bz_hi, bz_lo = _split_bf16(bz)

    pos_mask = tuple(bool(v) for v in (w2 >= 0))
    slots, _ = _slot_list(pos_mask)
    S = len(slots)
    units = _unit_list()

    in_maps = []
    for core in range(NCORES):
        A1g = np.zeros((4, S, UNITS_PER_CORE * P), dtype=ml_dtypes.bfloat16)
        B1g = np.zeros((4, S, UNITS_PER_CORE * F), dtype=ml_dtypes.bfloat16)
        for u in range(UNITS_PER_CORE):
            bi, col0 = units[core * UNITS_PER_CORE + u]
            for s, k in enumerate(slots):
                if k is None:
                    continue
                A1g[0, s, u * P:(u + 1) * P] = az_hi[k, bi * P:(bi + 1) * P]
                A1g[1, s, u * P:(u + 1) * P] = az_lo[k, bi * P:(bi + 1) * P]
                A1g[2, s, u * P:(u + 1) * P] = 1.0
                A1g[3, s, u * P:(u + 1) * P] = 1.0
                B1g[0, s, u * F:(u + 1) * F] = 1.0
                B1g[1, s, u * F:(u + 1) * F] = 1.0
                B1g[2, s, u * F:(u + 1) * F] = bz_hi[k, col0:col0 + F]
                B1g[3, s, u * F:(u + 1) * F] = bz_lo[k, col0:col0 + F]
        in_maps.append({
            "A1g": A1g,
            "B1g": B1g,
            "b2c": np.full((P, 1), b2[0], dtype=np.float32),
        })
    return in_maps, pos_mask, units


def kernel(loop_embeddings, W1, b1, W2, b2):
    from concourse.bass_utils import run_bass_kernel_spmd

    in_maps, pos_mask, units = _prep_inputs(loop_embeddings, W1, b1, W2, b2)

    if pos_mask not in _cache:
        _cache[pos_mask] = _build_module(pos_mask)
    nc = _cache[pos_mask]

    res = run_bass_kernel_spmd(nc, in_maps, list(range(NCORES)))

    s = np.zeros((N, N), dtype=np.float32)
    for core in range(NCORES):
        o = res.results[core]["out"]
        for u in range(UNITS_PER_CORE):
            bi, col0 = units[core * UNITS_PER_CORE + u]
            s[bi * P:(bi + 1) * P, col0:col0 + F] = o[u]
    up = np.triu(s, 1)
    return (up + up.T).astype(np.float32)
